# revision 18
# baseline (speedup 1.0000x reference)
"""Causal self-attention (B=4, T=2048, C=1024, H=16) on 8 TRN2 NeuronCores.

Sharding: 8 cores = 4 batches x 2 head-groups (8 heads each). Core c = g*4+b
handles batch b, heads 8g..8g+8 (4 pairs of 2). Inside kernel(): the host
transposes x[b] -> xT [C,T] (bf16), slices/arranges W_attn columns (Wq
pre-scaled by 1/sqrt(D)) and W_proj rows per group (bf16), runs one Bass/Tile
kernel SPMD on cores 0-7, then sums the two group-partial out^T [C,T] per
batch and transposes.

Per-core device pipeline (all matmuls bf16, PSUM f32), fused over 4 query
strips of 512 tokens so projection, attention, and output projection overlap:
  strip qs: 1. project Q/K (pair-packed [128,512]) and V for these tokens
            2. per (pair, head): key tiles jt=0..4qs+3 with exact causal
               spans; scores chunks (<=1024 cols, 2 key tiles packed) -> one
               ACT exp per chunk -> bf16 es -> diagonal mask-mul (DVE) ->
               PV accumulate [V|1]^T @ es into y_ps [65,512] (1 PSUM bank)
            3. normalize: recip (DVE) -> partition_broadcast (Pool) -> mul
               (DVE) -> ysb bf16
            4. (after proj of strip qs+1, to hide the normalize tail)
               output projection of strip qs -> DMA out
"""
import sys
if '/opt/trn_rl_repo' not in sys.path:
    sys.path.insert(0, '/opt/trn_rl_repo')
import numpy as np
import concourse.bacc as bacc
import concourse.tile as tile
import concourse.mybir as mybir
from concourse import bass_utils

F32 = mybir.dt.float32
BF16 = mybir.dt.bfloat16
NP_BF16 = mybir.dt.np(BF16)

N_EMBED = 1024
N_HEAD = 16
D = 64
B_FULL, T_FULL, C_FULL = 4, 2048, 1024
N_GROUPS = 2


def build_kernel(T=T_FULL, C=C_FULL, n_pairs=4, reps=1, n_strip=512, abl=()):
    abl = frozenset(abl)
    HP = n_pairs * 2            # heads per core
    CIN = HP * D                # 512
    n_k = C // 128              # contraction tiles for projections
    n_jt = T // 128             # key tiles
    n_ts = T // n_strip         # query strips
    jt_per_strip = n_strip // 128

    nc = bacc.Bacc("TRN2", target_bir_lowering=False, debug=False)
    xt_d = nc.dram_tensor("xt", [C, T], BF16, kind="ExternalInput")
    wqk_d = nc.dram_tensor("wqk", [C, n_pairs * 256], BF16, kind="ExternalInput")
    wv_d = nc.dram_tensor("wv", [C, n_pairs * 128], BF16, kind="ExternalInput")
    wp_d = nc.dram_tensor("wp", [CIN, C], BF16, kind="ExternalInput")
    mask_d = nc.dram_tensor("mask", [128, 256], BF16, kind="ExternalInput")
    outp_d = nc.dram_tensor("outp", [C, T], BF16, kind="ExternalOutput")

    xt_r = xt_d.ap().rearrange("(k p) t -> p k t", p=128)
    wqk_r = wqk_d.ap().rearrange("(k p) m -> p k m", p=128)
    wv_r = wv_d.ap().rearrange("(k p) m -> p k m", p=128)
    wp_r = wp_d.ap().rearrange("(k p) m -> p k m", p=128)

    with tile.TileContext(nc) as tc:
        with tc.tile_pool(name="wqkp", bufs=1) as wqkp, \
             tc.tile_pool(name="wvp", bufs=1) as wvp, \
             tc.tile_pool(name="wpp", bufs=1) as wpp, \
             tc.tile_pool(name="xsp", bufs=2) as xsp, \
             tc.tile_pool(name="qkp", bufs=1) as qkp, \
             tc.tile_pool(name="vp", bufs=1) as vp, \
             tc.tile_pool(name="maskp", bufs=1) as maskp, \
             tc.tile_pool(name="esp", bufs=3) as esp, \
             tc.tile_pool(name="normp", bufs=2) as normp, \
             tc.tile_pool(name="ysbp", bufs=1) as ysbp, \
             tc.tile_pool(name="osbp", bufs=4) as osbp, \
             tc.tile_pool(name="ps_a", bufs=2, space="PSUM") as ps_a, \
             tc.tile_pool(name="ps_s", bufs=2, space="PSUM") as ps_s, \
             tc.tile_pool(name="ps_y", bufs=2, space="PSUM") as ps_y:

            def body(_i=None):
                cp_eng = nc.scalar.copy if "actcopies" in abl else nc.vector.tensor_copy
                # x strip 0 first (critical path), then QK weights per pair,
                # then the rest in order of first use.
                xs0 = xsp.tile([128, n_k, n_strip], BF16, tag="xs")
                nc.sync.dma_start(out=xs0[:, 0:1, :], in_=xt_r[:, 0:1, 0:n_strip])
                wqk_sb = wqkp.tile([128, n_k, n_pairs * 256], BF16, tag="wqk")
                nc.sync.dma_start(out=wqk_sb[:, 0:3, 0:256], in_=wqk_r[:, 0:3, 0:256])
                nc.sync.dma_start(out=xs0[:, 1:3, :], in_=xt_r[:, 1:3, 0:n_strip])
                nc.sync.dma_start(out=wqk_sb[:, 3:n_k, 0:256], in_=wqk_r[:, 3:n_k, 0:256])
                nc.sync.dma_start(out=xs0[:, 3:n_k, :], in_=xt_r[:, 3:n_k, 0:n_strip])
                for p in range(1, n_pairs):
                    nc.sync.dma_start(
                        out=wqk_sb[:, :, p * 256:(p + 1) * 256],
                        in_=wqk_r[:, :, p * 256:(p + 1) * 256])
                wv_sb = wvp.tile([128, n_k, n_pairs * 128], BF16, tag="wv")
                nc.sync.dma_start(out=wv_sb[:], in_=wv_r)
                mask_sb = maskp.tile([128, 256], BF16, tag="mask")
                nc.sync.dma_start(out=mask_sb[:], in_=mask_d.ap())
                wp_sb = wpp.tile([128, CIN // 128, C], BF16, tag="wp")
                nc.sync.dma_start(out=wp_sb[:], in_=wp_r)

                qt = [qkp.tile([128, T], BF16, tag=f"qt{p}", name=f"qt{p}")
                      for p in range(n_pairs)]
                kt = [qkp.tile([128, T], BF16, tag=f"kt{p}", name=f"kt{p}")
                      for p in range(n_pairs)]
                v_aug = vp.tile([128, n_jt, HP, 65], BF16, tag="vaug")
                nc.vector.memset(v_aug[:], 1.0)
                ysb = ysbp.tile([128, n_pairs, T], BF16, tag="ysb")

                def proj_units(qs, xs):
                    """Yield the 12 projection work units for strip qs."""
                    sl = slice(qs * n_strip, (qs + 1) * n_strip)

                    def qk_unit(p, qk):
                        ps = ps_a.tile([128, n_strip], F32, tag="a")
                        for k in range(n_k):
                            if "projmm" in abl: break
                            nc.tensor.matmul(
                                ps[:],
                                wqk_sb[:, k, (p * 2 + qk) * 128:(p * 2 + qk + 1) * 128],
                                xs[:, k, :],
                                start=(k == 0), stop=(k == n_k - 1))
                        dst = (qt if qk == 0 else kt)[p]
                        if "copies" not in abl:
                            cp_eng(dst[:, sl], ps[:])

                    def v_unit(nt):
                        psv = ps_a.tile([128, n_pairs * 128], F32, tag="a")
                        for k in range(n_k):
                            if "projmm" in abl: break
                            nc.tensor.matmul(
                                psv[:], xs[:, k, nt * 128:(nt + 1) * 128],
                                wv_sb[:, k, :],
                                start=(k == 0), stop=(k == n_k - 1))
                        jt = qs * jt_per_strip + nt
                        if "copies" not in abl:
                            cp_eng(
                                v_aug[:, jt, :, 0:64],
                                psv[:].rearrange("q (h d) -> q h d", d=D))

                    for p in range(n_pairs):
                        for qk in range(2):
                            yield lambda p=p, qk=qk: qk_unit(p, qk)
                    for nt in range(jt_per_strip):
                        yield lambda nt=nt: v_unit(nt)

                def outproj_units(qs):
                    """Yield the 8 output-projection units for strip qs."""
                    sl = slice(qs * n_strip, (qs + 1) * n_strip)

                    def m_unit(m):
                        pso = ps_a.tile([128, n_strip], F32, tag="a")
                        for kp in range(CIN // 128):
                            if "outprojmm" in abl: break
                            nc.tensor.matmul(
                                pso[:],
                                wp_sb[:, kp, m * 128:(m + 1) * 128],
                                ysb[:, kp, sl],
                                start=(kp == 0), stop=(kp == CIN // 128 - 1))
                        osb = osbp.tile([128, n_strip], BF16, tag="osb")
                        nc.scalar.copy(osb[:], pso[:])
                        nc.sync.dma_start(
                            out=outp_d.ap()[m * 128:(m + 1) * 128, sl], in_=osb[:])

                    for m in range(C // 128):
                        yield lambda m=m: m_unit(m)

                def attention(qs, filler):
                    q0 = qs * n_strip
                    n_jt_q = (qs + 1) * jt_per_strip
                    # (jt, seg offset within strip, width)
                    segs = []
                    for jt in range(n_jt_q):
                        a = max(0, 128 * jt - q0)
                        segs.append((jt, a, n_strip - a))
                    # pack consecutive segs into chunks of <= 1024 cols
                    chunks = []
                    cur, cw = [], 0
                    for s in segs:
                        if cw + s[2] > 1024:
                            chunks.append(cur)
                            cur, cw = [], 0
                        cur.append(s)
                        cw += s[2]
                    if cur:
                        chunks.append(cur)
                    n_heads = 2 * n_pairs
                    emitted = 0
                    n_pos = n_heads * len(chunks)
                    pos = 0
                    for p in range(n_pairs):
                        for h in range(2):
                            hh = p * 2 + h
                            hs = slice(h * 64, (h + 1) * 64)
                            y_ps = ps_y.tile([65, n_strip], F32, tag="y")
                            for chunk in chunks:
                                sc = ps_s.tile([128, 1024], F32, tag="sc")
                                off = 0
                                offs = []
                                for (jt, a, w) in chunk:
                                    if "scoremm" in abl:
                                        offs.append(off); off += w
                                        continue
                                    diag = jt >= qs * jt_per_strip and "mask" not in abl
                                    nc.tensor.matmul(
                                        sc[:, off:off + w],
                                        kt[p][hs, 128 * jt:128 * jt + 128],
                                        qt[p][hs, q0 + a:q0 + n_strip],
                                        start=True, stop=not diag,
                                        skip_group_check=True)
                                    if diag:
                                        # accumulate -1e9 onto the strictly
                                        # upper triangle of the diagonal block
                                        # (identity.T @ negtri) so exp gives 0
                                        nc.tensor.matmul(
                                            sc[:, off:off + 128],
                                            mask_sb[:, 128:256],
                                            mask_sb[:, 0:128],
                                            start=False, stop=True,
                                            skip_group_check=True)
                                    offs.append(off)
                                    off += w
                                es = esp.tile([128, 1024], BF16, tag="es")
                                if "exp" not in abl: nc.scalar.activation(
                                    out=es[:, 0:off], in_=sc[:, 0:off],
                                    func=mybir.ActivationFunctionType.Exp)
                                for (jt, a, w), o in zip(chunk, offs):
                                    if "pvmm" in abl: continue
                                    nc.tensor.matmul(
                                        y_ps[:, a:n_strip],
                                        v_aug[:, jt, hh, :],
                                        es[:, o:o + w],
                                        start=(jt == 0),
                                        stop=(jt == n_jt_q - 1),
                                        skip_group_check=True)
                                # interleave independent proj/outproj work at
                                # chunk granularity so PE stays fed while ACT
                                # catches up on exp
                                pos += 1
                                want = len(filler) * pos // n_pos
                                while emitted < want:
                                    filler[emitted]()
                                    emitted += 1
                            if "norm" not in abl:
                                recip = normp.tile([1, n_strip], F32, tag="recip")
                                if "actrecip" in abl:
                                    nc.scalar.activation(
                                        out=recip[:], in_=y_ps[64:65, :],
                                        func=mybir.ActivationFunctionType.Reciprocal)
                                else:
                                    nc.vector.reciprocal(recip[:], y_ps[64:65, :])
                                if "pbap" in abl:
                                    nc.vector.tensor_mul(
                                        ysb[h * 64:(h + 1) * 64, p, q0:q0 + n_strip],
                                        y_ps[0:64, :],
                                        recip[:].partition_broadcast(64))
                                else:
                                    bcast = normp.tile([64, n_strip], F32, tag="bcast")
                                    nc.gpsimd.partition_broadcast(bcast[:], recip[:])
                                    nc.vector.tensor_mul(
                                        ysb[h * 64:(h + 1) * 64, p, q0:q0 + n_strip],
                                        y_ps[0:64, :], bcast[:])
                    while emitted < len(filler):
                        filler[emitted]()
                        emitted += 1

                # strip 0 projection runs alone (nothing to overlap with).
                # Output projections all pile into the last strip's attention
                # (the only ACT-bound phase needing that much PE filler).
                for u in proj_units(0, xs0):
                    u()
                xs_next = None
                for qs in range(n_ts):
                    filler = []
                    if qs + 1 < n_ts:
                        xs_next = xsp.tile([128, n_k, n_strip], BF16, tag="xs")
                        nc.sync.dma_start(
                            out=xs_next[:],
                            in_=xt_r[:, :, (qs + 1) * n_strip:(qs + 2) * n_strip])
                        filler.extend(proj_units(qs + 1, xs_next))
                    else:
                        for oq in range(n_ts - 1):
                            filler.extend(outproj_units(oq))
                    attention(qs, filler)
                for u in outproj_units(n_ts - 1):
                    u()

            if reps == 1:
                body()
            else:
                with tc.For_i(0, reps, 1) as i:
                    body(i)
    nc.compile()
    return nc


def host_inputs(x, W_attn, W_proj, n_groups=N_GROUPS):
    """Per-core input maps. Core order: g * B + b."""
    B, T, C = x.shape
    hp = N_HEAD // n_groups
    n_pairs = hp // 2
    scale = np.float32(1.0 / np.sqrt(D))
    negtri = np.where(np.arange(128)[None, :] < np.arange(128)[:, None],
                      np.float32(-1e9), np.float32(0.0))
    mask = np.concatenate([negtri, np.eye(128, dtype=np.float32)],
                          axis=1).astype(NP_BF16)
    in_maps = []
    for g in range(n_groups):
        qk_cols, v_cols = [], []
        for p in range(n_pairs):
            h0 = g * hp + 2 * p
            h1 = h0 + 1
            qk_cols.append(W_attn[:, h0 * D:(h0 + 1) * D] * scale)
            qk_cols.append(W_attn[:, h1 * D:(h1 + 1) * D] * scale)
            qk_cols.append(W_attn[:, C + h0 * D:C + (h0 + 1) * D])
            qk_cols.append(W_attn[:, C + h1 * D:C + (h1 + 1) * D])
            v_cols.append(W_attn[:, 2 * C + h0 * D:2 * C + (h0 + 1) * D])
            v_cols.append(W_attn[:, 2 * C + h1 * D:2 * C + (h1 + 1) * D])
        wqk = np.ascontiguousarray(
            np.concatenate(qk_cols, axis=1)).astype(NP_BF16)
        wv = np.ascontiguousarray(
            np.concatenate(v_cols, axis=1)).astype(NP_BF16)
        wp = np.ascontiguousarray(
            W_proj[g * hp * D:(g + 1) * hp * D]).astype(NP_BF16)
        for b in range(B):
            xt = np.ascontiguousarray(x[b].T).astype(NP_BF16)
            in_maps.append({"xt": xt, "wqk": wqk, "wv": wv, "wp": wp, "mask": mask})
    return in_maps


def host_gather(results, B, T, C, n_groups=N_GROUPS):
    out = np.zeros((B, T, C), dtype=np.float32)
    for g in range(n_groups):
        for b in range(B):
            out[b] += results[g * B + b]["outp"].T.astype(np.float32)
    return out


_NC_CACHE = {}


def kernel(x, W_attn, W_proj):
    x = np.asarray(x, dtype=np.float32)
    W_attn = np.asarray(W_attn, dtype=np.float32)
    W_proj = np.asarray(W_proj, dtype=np.float32)
    B, T, C = x.shape
    if "nc" not in _NC_CACHE:
        _NC_CACHE["nc"] = build_kernel(T=T, C=C)
    nc = _NC_CACHE["nc"]
    in_maps = host_inputs(x, W_attn, W_proj)
    res = bass_utils.run_bass_kernel_spmd(nc, in_maps, core_ids=list(range(8)))
    return host_gather(res.results, B, T, C)
